# revision 6
# baseline (speedup 1.0000x reference)
"""GCN encoder (2-layer graph conv) on 8 Trainium2 NeuronCores.

Strategy (sharding_hint): nodes sharded across the 8 cores by destination row
(contiguous blocks of 6272 padded rows); edges partitioned by destination;
256x256 weights replicated. Per layer:

  1. each core computes its shard of support = x @ W (dense matmul, fp16),
  2. AllGather -> full fp16 "support table" [50176, 256] in every core's HBM,
  3. per 128-row destination tile: dma_gather the source rows for its edges
     (sorted by dest, sub-grouped by source half so int16 indices fit),
     build a scaled one-hot selector S[e, dest_local] = val_e on the DVE
     (one tensor_scalar: (iota == dest) * val), and accumulate
     agg += S^T-style matmuls into PSUM (segment-sum as matmul),
  4. epilogue relu(agg + b) on the scalar engine.

Layer 1 runs the selector matmul "transposed" (out = G_half^T @ S -> [feat,
dest]) so the result lands as hT, which feeds layer 2's support matmul as the
stationary operand without any transposes. Layer 2 runs it straight
(out = S^T @ G -> [dest, feat]) so the final output is row-major.

Host-side work is only index/format preprocessing (sort edges, build the
padded chunk metadata) and the final unshard/pad via pos_idx.
"""

import os
import sys

if "/opt/trn_rl_repo" not in sys.path:
    sys.path.insert(0, "/opt/trn_rl_repo")

import numpy as np

import concourse.mybir as mybir
import concourse.tile as tile
from concourse import bacc, bass_utils
from concourse.bass import ts
from concourse.library_config import mlp

# Problem geometry (nn_GCNEncoder: N=50000, E=1.6M, 256 features, pad to 60000)
N = 50000
D = 256
NC = 8
P = 128
T = 49  # dest-row tiles per core
SHARD = T * P  # 6272 rows per core
NPAD = NC * SHARD  # 50176
HALF = NPAD // 2  # 25088 (< 2**15, so int16 gather indices fit per half)

F16 = mybir.dt.float16
F32 = mybir.dt.float32
I16 = mybir.dt.int16

_cache: dict = {}
last_results = None  # BassKernelResults of the most recent run (for profiling)


def _build(C0: int, C1: int, trace_scopes: bool = False):
    """Build + compile the SPMD program for uniform per-(tile, source-half)
    chunk counts C0/C1 (chunks of 128 edges)."""
    key = (C0, C1)
    if key in _cache:
        return _cache[key]

    C = C0 + C1
    nc = bacc.Bacc("TRN2", target_bir_lowering=False, debug=False, num_devices=NC)

    xT_d = nc.dram_tensor("xT", [2, P, SHARD], F16, kind="ExternalInput")
    w1_d = nc.dram_tensor("W1h", [2, P, D], F16, kind="ExternalInput")
    w2_d = nc.dram_tensor("W2h", [2, P, D], F16, kind="ExternalInput")
    b1_d = nc.dram_tensor("b1c", [P, 2], F32, kind="ExternalInput")
    b2_d = nc.dram_tensor("b2b", [P, D], F32, kind="ExternalInput")
    iota_d = nc.dram_tensor("iota128", [P, P], F16, kind="ExternalInput")
    gidx_d = nc.dram_tensor("gidx", [P, T * C * 8], I16, kind="ExternalInput")
    meta_d = nc.dram_tensor("meta", [P, T * 2 * C], F32, kind="ExternalInput")
    out_d = nc.dram_tensor("out", [SHARD, D], F32, kind="ExternalOutput")

    nc.gpsimd.load_library(mlp)

    rg = [list(range(NC))]

    with tile.TileContext(nc) as tc:
        with (
            tc.tile_pool(name="const", bufs=1) as const,
            tc.tile_pool(name="gpool", bufs=3) as gpool,
            tc.tile_pool(name="spool", bufs=4) as spool,
            tc.tile_pool(name="dense", bufs=3) as dense,
            tc.tile_pool(name="psD", bufs=2, space="PSUM") as psD,
            tc.tile_pool(name="psA", bufs=2, space="PSUM") as psA,
            tc.tile_pool(name="dram", bufs=1, space="DRAM") as dram,
        ):
            cc1_in = dram.tile([SHARD, D], F16)
            table1 = dram.tile([NPAD, D], F16, addr_space="Shared")
            cc2_in = dram.tile([SHARD, D], F16)
            table2 = dram.tile([NPAD, D], F16, addr_space="Shared")

            # --- persistent SBUF state ---
            gidx = const.tile([P, T * C * 8], I16)
            nc.sync.dma_start(gidx[:], gidx_d[:])
            meta = const.tile([P, T * 2 * C], F32)
            nc.sync.dma_start(meta[:], meta_d[:])
            iota = const.tile([P, P], F16)
            nc.sync.dma_start(iota[:], iota_d[:])
            b1 = const.tile([P, 2], F32)
            nc.sync.dma_start(b1[:], b1_d[:])
            b2 = const.tile([P, D], F32)
            nc.sync.dma_start(b2[:], b2_d[:])
            w1 = const.tile([P, 2 * D], F16)
            w2 = const.tile([P, 2 * D], F16)
            xsb = const.tile([P, 2 * SHARD], F16)
            hT = const.tile([P, 2 * SHARD], F16)
            for h in range(2):
                nc.sync.dma_start(w1[:, h * D : (h + 1) * D], w1_d[h])
                nc.sync.dma_start(w2[:, h * D : (h + 1) * D], w2_d[h])
                nc.sync.dma_start(xsb[:, h * SHARD : (h + 1) * SHARD], xT_d[h])

            def dense_support(src_sb, w_sb, cc_in):
                # support[rows, :] = x[rows, :] @ W  (K=256 split into 2 halves)
                for t in range(T):
                    ps = psD.tile([P, D], F32, tag="psD", name="ps_dense")
                    for h in range(2):
                        nc.tensor.matmul(
                            ps,
                            lhsT=src_sb[:, h * SHARD + t * P : h * SHARD + (t + 1) * P],
                            rhs=w_sb[:, h * D : (h + 1) * D],
                            start=(h == 0),
                            stop=(h == 1),
                        )
                    st = dense.tile([P, D], F16, tag="stage", name="stage")
                    nc.scalar.copy(st[:], ps[:])
                    nc.sync.dma_start(cc_in[ts(t, P), :], st[:])

            def gather_tile(table, t, tag):
                # dma_gather hard-crashes above 1024 indices -> <=8 chunks per op
                g = gpool.tile([P, C, D], F16, tag=tag, name="gtile")
                base = t * C * 8
                for lo, hi, toff in ((0, C0, 0), (C0, C, HALF)):
                    k = lo
                    while k < hi:
                        kk = min(8, hi - k)
                        nc.gpsimd.dma_gather(
                            g[:, k : k + kk, :],
                            table[toff : toff + HALF, :],
                            gidx[:, base + k * 8 : base + (k + kk) * 8],
                            num_idxs=kk * P,
                            num_idxs_reg=kk * P,
                            elem_size=D,
                        )
                        k += kk
                return g

            def selector(t, k):
                s_t = spool.tile([P, P], F16, tag="sel", name="sel")
                mcol = t * 2 * C + 2 * k
                nc.vector.tensor_scalar(
                    s_t[:],
                    iota[:],
                    meta[:, mcol : mcol + 1],
                    meta[:, mcol + 1 : mcol + 2],
                    mybir.AluOpType.is_equal,
                    mybir.AluOpType.mult,
                )
                return s_t

            # ---------- layer 1 ----------
            dense_support(xsb, w1, cc1_in)
            nc.gpsimd.collective_compute(
                "AllGather",
                mybir.AluOpType.bypass,
                replica_groups=rg,
                ins=[cc1_in.opt()],
                outs=[table1.opt()],
            )
            for t in range(T):
                g = gather_tile(table1, t, "g1")
                pss = [
                    psA.tile([P, P], F32, tag=f"agg{h}", name=f"agg{h}")
                    for h in range(2)
                ]
                for k in range(C):
                    s_t = selector(t, k)
                    for h in range(2):
                        # out[feat_h, dest] += G[:, k, feat_h]^T @ S
                        nc.tensor.matmul(
                            pss[h],
                            lhsT=g[:, k, h * P : (h + 1) * P],
                            rhs=s_t[:],
                            start=(k == 0),
                            stop=(k == C - 1),
                        )
                for h in range(2):
                    # hT[feat_h, t*128:...] = relu(agg + b1[feat_h])
                    nc.scalar.activation(
                        hT[:, h * SHARD + t * P : h * SHARD + (t + 1) * P],
                        pss[h][:],
                        mybir.ActivationFunctionType.Relu,
                        bias=b1[:, h : h + 1],
                    )

            # ---------- layer 2 ----------
            dense_support(hT, w2, cc2_in)
            nc.gpsimd.collective_compute(
                "AllGather",
                mybir.AluOpType.bypass,
                replica_groups=rg,
                ins=[cc2_in.opt()],
                outs=[table2.opt()],
            )
            for t in range(T):
                g = gather_tile(table2, t, "g2")
                ps = psD.tile([P, D], F32, tag="psD", name="agg2")
                for k in range(C):
                    s_t = selector(t, k)
                    # out[dest, :] += S^T @ G[:, k, :]
                    nc.tensor.matmul(
                        ps,
                        lhsT=s_t[:],
                        rhs=g[:, k, :],
                        start=(k == 0),
                        stop=(k == C - 1),
                    )
                nc.vector.tensor_tensor(ps[:], ps[:], b2[:], mybir.AluOpType.add)
                ot = dense.tile([P, D], F32, tag="ot", name="ot")
                nc.scalar.activation(
                    ot[:], ps[:], mybir.ActivationFunctionType.Relu
                )
                nc.sync.dma_start(out_d[ts(t, P), :], ot[:])

    nc.compile()
    _cache[key] = nc
    return nc


def _wrap_idx16(flat: np.ndarray) -> np.ndarray:
    """[L] int -> [128, L/16] int16 SBUF wrap: sb[p, s] = flat[s*16 + p%16]."""
    L = flat.shape[0]
    base = flat.reshape(L // 16, 16).T.astype(np.int16)  # [16, L/16]
    return np.tile(base, (8, 1))


def _preprocess(adj_rows, adj_cols, adj_vals):
    r = np.asarray(adj_rows).astype(np.int64)
    c = np.asarray(adj_cols).astype(np.int64)
    v = np.asarray(adj_vals).astype(np.float32)
    E = r.shape[0]

    core = r // SHARD
    tile_id = (r % SHARD) // P
    dest_local = (r % P).astype(np.float32)
    half = (c >= HALF).astype(np.int64)
    idx_local = c - half * HALF

    key = (core * T + tile_id) * 2 + half
    order = np.lexsort((c, key))
    key_s = key[order]
    counts = np.bincount(key_s, minlength=NC * T * 2)
    grp_start = np.concatenate(([0], np.cumsum(counts)))[:-1]
    j = np.arange(E) - grp_start[key_s]

    cnt = counts.reshape(NC, T, 2)
    C0 = max(1, -(-int(cnt[:, :, 0].max()) // P))
    C1 = -(-int(cnt[:, :, 1].max()) // P)
    C = C0 + C1

    core_s = core[order]
    tile_s = tile_id[order]
    half_s = half[order]
    chunk = j // P + np.where(half_s == 1, C0, 0)
    part = j % P

    idx_pad = np.zeros((NC, T, C, P), np.int16)
    dest_pad = np.zeros((NC, T, C, P), np.float32)
    val_pad = np.zeros((NC, T, C, P), np.float32)
    idx_pad[core_s, tile_s, chunk, part] = idx_local[order].astype(np.int16)
    dest_pad[core_s, tile_s, chunk, part] = dest_local[order]
    val_pad[core_s, tile_s, chunk, part] = v[order]

    # gidx: per (core, tile): C0*8 cols of wrapped group-0 idxs, then C1*8
    gidx = np.zeros((NC, P, T * C * 8), np.int16)
    for cr in range(NC):
        for t in range(T):
            base = t * C * 8
            g0 = idx_pad[cr, t, 0:C0, :].reshape(C0 * P)
            gidx[cr, :, base : base + C0 * 8] = _wrap_idx16(g0)
            if C1 > 0:
                g1 = idx_pad[cr, t, C0:C, :].reshape(C1 * P)
                gidx[cr, :, base + C0 * 8 : base + C * 8] = _wrap_idx16(g1)

    # meta: [P, T*2C] with cols t*2C + 2k = dest_local, +1 = val
    meta = np.empty((NC, P, T, C, 2), np.float32)
    meta[..., 0] = dest_pad.transpose(0, 3, 1, 2)
    meta[..., 1] = val_pad.transpose(0, 3, 1, 2)
    meta = meta.reshape(NC, P, T * C * 2)

    return C0, C1, gidx, meta


def kernel(
    x, adj_rows, adj_cols, adj_vals, pad_n, pos_idx, W1, b1, W2, b2
) -> np.ndarray:
    x = np.asarray(x, np.float32)
    W1 = np.asarray(W1, np.float32)
    b1 = np.asarray(b1, np.float32)
    W2 = np.asarray(W2, np.float32)
    b2 = np.asarray(b2, np.float32)
    pos_idx = np.asarray(pos_idx).astype(np.int64)
    pad_n_i = int(pad_n)
    assert x.shape == (N, D)

    C0, C1, gidx, meta = _preprocess(adj_rows, adj_cols, adj_vals)
    nc = _build(C0, C1)

    xpad = np.zeros((NPAD, D), np.float32)
    xpad[:N] = x
    w1h = W1.astype(np.float16).reshape(2, P, D)
    w2h = W2.astype(np.float16).reshape(2, P, D)
    b1c = np.ascontiguousarray(b1.reshape(2, P).T.astype(np.float32))
    b2b = np.ascontiguousarray(np.broadcast_to(b2, (P, D)).astype(np.float32))
    iota128 = np.ascontiguousarray(
        np.broadcast_to(np.arange(P, dtype=np.float16), (P, P))
    )

    in_maps = []
    for cr in range(NC):
        xT = np.ascontiguousarray(
            xpad[cr * SHARD : (cr + 1) * SHARD].T.astype(np.float16).reshape(2, P, SHARD)
        )
        in_maps.append(
            {
                "xT": xT,
                "W1h": w1h,
                "W2h": w2h,
                "b1c": b1c,
                "b2b": b2b,
                "iota128": iota128,
                "gidx": np.ascontiguousarray(gidx[cr]),
                "meta": np.ascontiguousarray(meta[cr]),
            }
        )

    trace = bool(int(os.environ.get("KERNEL_TRACE", "0")))
    res = None
    for attempt in range(3):
        try:
            res = bass_utils.run_bass_kernel_spmd(
                nc, in_maps, core_ids=list(range(NC)), trace=trace
            )
            break
        except Exception:
            if attempt == 2:
                raise
            import time as _time

            _time.sleep(10.0)
    global last_results
    last_results = res

    h2 = np.concatenate([res.results[cr]["out"] for cr in range(NC)], axis=0)[:N]
    out = np.zeros((pad_n_i, D), np.float32)
    out[pos_idx] = h2
    return out


# revision 8
# speedup vs baseline: 1.6353x; 1.6353x over previous
"""GCN encoder (2-layer graph conv) on 8 Trainium2 NeuronCores.

Strategy (sharding_hint): nodes sharded across the 8 cores by destination row
(contiguous blocks of 6272 padded rows); edges partitioned by destination;
256x256 weights replicated. Per layer:

  1. each core computes its shard of support = x @ W (dense matmul, fp16),
  2. AllGather -> full fp16 "support table" [50176, 256] in every core's HBM,
  3. per 128-row destination tile: dma_gather the source rows for its edges
     (sorted by dest, sub-grouped by source half so int16 indices fit),
     build a scaled one-hot selector S[e, dest_local] = val_e on the DVE
     (one tensor_scalar: (iota == dest) * val), and accumulate
     agg += S^T-style matmuls into PSUM (segment-sum as matmul),
  4. epilogue relu(agg + b) on the scalar engine.

Layer 1 runs the selector matmul "transposed" (out = G_half^T @ S -> [feat,
dest]) so the result lands as hT, which feeds layer 2's support matmul as the
stationary operand without any transposes. Layer 2 runs it straight
(out = S^T @ G -> [dest, feat]) so the final output is row-major.

Host-side work is only index/format preprocessing (sort edges, build the
padded chunk metadata) and the final unshard/pad via pos_idx.
"""

import os
import sys

if "/opt/trn_rl_repo" not in sys.path:
    sys.path.insert(0, "/opt/trn_rl_repo")

import numpy as np

import concourse.mybir as mybir
import concourse.tile as tile
from concourse import bacc, bass_utils
from concourse.bass import ts
from concourse.library_config import mlp

# Problem geometry (nn_GCNEncoder: N=50000, E=1.6M, 256 features, pad to 60000)
N = 50000
D = 256
NC = 8
P = 128
T = 49  # dest-row tiles per core
SHARD = T * P  # 6272 rows per core
NPAD = NC * SHARD  # 50176
HALF = NPAD // 2  # 25088 (< 2**15, so int16 gather indices fit per half)

F16 = mybir.dt.float16
F32 = mybir.dt.float32
I16 = mybir.dt.int16

_cache: dict = {}
last_results = None  # BassKernelResults of the most recent run (for profiling)


def _build(C0: int, C1: int, trace_scopes: bool = False):
    """Build + compile the SPMD program for uniform per-(tile, source-half)
    chunk counts C0/C1 (chunks of 128 edges)."""
    key = (C0, C1)
    if key in _cache:
        return _cache[key]

    C = C0 + C1
    nc = bacc.Bacc(
        "TRN2",
        target_bir_lowering=False,
        debug=False,
        num_devices=NC,
        num_swdge_queues=4,
    )

    xT_d = nc.dram_tensor("xT", [2, P, SHARD], F16, kind="ExternalInput")
    w1_d = nc.dram_tensor("W1h", [2, P, D], F16, kind="ExternalInput")
    w2_d = nc.dram_tensor("W2h", [2, P, D], F16, kind="ExternalInput")
    b1_d = nc.dram_tensor("b1c", [P, 2], F32, kind="ExternalInput")
    b2_d = nc.dram_tensor("b2b", [P, D], F32, kind="ExternalInput")
    iota_d = nc.dram_tensor("iota128", [P, P], F16, kind="ExternalInput")
    gidx_d = nc.dram_tensor("gidx", [P, T * C * 8], I16, kind="ExternalInput")
    meta_d = nc.dram_tensor("meta", [P, T * 2 * C], F32, kind="ExternalInput")
    out_d = nc.dram_tensor("out", [SHARD, D], F32, kind="ExternalOutput")

    nc.gpsimd.load_library(mlp)

    rg = [list(range(NC))]

    with tile.TileContext(nc) as tc:
        with (
            tc.tile_pool(name="const", bufs=1) as const,
            tc.tile_pool(name="gpool", bufs=3) as gpool,
            tc.tile_pool(name="spool", bufs=4) as spool,
            tc.tile_pool(name="dense", bufs=3) as dense,
            tc.tile_pool(name="psD", bufs=2, space="PSUM") as psD,
            tc.tile_pool(name="psA", bufs=2, space="PSUM") as psA,
            tc.tile_pool(name="dram", bufs=1, space="DRAM") as dram,
        ):
            cc1_in = dram.tile([SHARD, D], F16)
            table1 = dram.tile([NPAD, D], F16, addr_space="Shared")
            cc2_in = dram.tile([SHARD, D], F16)
            table2 = dram.tile([NPAD, D], F16, addr_space="Shared")

            # --- persistent SBUF state ---
            gidx = const.tile([P, T * C * 8], I16)
            nc.sync.dma_start(gidx[:], gidx_d[:])
            meta = const.tile([P, T * 2 * C], F32)
            nc.sync.dma_start(meta[:], meta_d[:])
            iota = const.tile([P, P], F16)
            nc.sync.dma_start(iota[:], iota_d[:])
            b1 = const.tile([P, 2], F32)
            nc.sync.dma_start(b1[:], b1_d[:])
            b2 = const.tile([P, D], F32)
            nc.sync.dma_start(b2[:], b2_d[:])
            w1 = const.tile([P, 2 * D], F16)
            w2 = const.tile([P, 2 * D], F16)
            xsb = const.tile([P, 2 * SHARD], F16)
            hT = const.tile([P, 2 * SHARD], F16)
            for h in range(2):
                nc.sync.dma_start(w1[:, h * D : (h + 1) * D], w1_d[h])
                nc.sync.dma_start(w2[:, h * D : (h + 1) * D], w2_d[h])
                nc.sync.dma_start(xsb[:, h * SHARD : (h + 1) * SHARD], xT_d[h])

            def dense_support(src_sb, w_sb, cc_in):
                # support[rows, :] = x[rows, :] @ W  (K=256 split into 2 halves)
                for t in range(T):
                    ps = psD.tile([P, D], F32, tag="psD", name="ps_dense")
                    for h in range(2):
                        nc.tensor.matmul(
                            ps,
                            lhsT=src_sb[:, h * SHARD + t * P : h * SHARD + (t + 1) * P],
                            rhs=w_sb[:, h * D : (h + 1) * D],
                            start=(h == 0),
                            stop=(h == 1),
                        )
                    st = dense.tile([P, D], F16, tag="stage", name="stage")
                    nc.scalar.copy(st[:], ps[:])
                    nc.sync.dma_start(cc_in[ts(t, P), :], st[:])

            gather_counter = [0]

            def gather_tile(table, t, tag):
                # dma_gather hard-crashes above 1024 indices -> <=8 chunks per op;
                # round-robin across the 4 SWDGE queues for parallel desc-gen
                g = gpool.tile([P, C, D], F16, tag=tag, name="gtile")
                base = t * C * 8
                for lo, hi, toff in ((0, C0, 0), (C0, C, HALF)):
                    k = lo
                    while k < hi:
                        kk = min(8, hi - k)
                        nc.gpsimd.dma_gather(
                            g[:, k : k + kk, :],
                            table[toff : toff + HALF, :],
                            gidx[:, base + k * 8 : base + (k + kk) * 8],
                            num_idxs=kk * P,
                            num_idxs_reg=kk * P,
                            elem_size=D,
                            queue_num=gather_counter[0] % 4,
                        )
                        gather_counter[0] += 1
                        k += kk
                return g

            def selector(t, k):
                s_t = spool.tile([P, P], F16, tag="sel", name="sel")
                mcol = t * 2 * C + 2 * k
                nc.vector.tensor_scalar(
                    s_t[:],
                    iota[:],
                    meta[:, mcol : mcol + 1],
                    meta[:, mcol + 1 : mcol + 2],
                    mybir.AluOpType.is_equal,
                    mybir.AluOpType.mult,
                )
                return s_t

            # ---------- layer 1 ----------
            dense_support(xsb, w1, cc1_in)
            nc.gpsimd.collective_compute(
                "AllGather",
                mybir.AluOpType.bypass,
                replica_groups=rg,
                ins=[cc1_in.opt()],
                outs=[table1.opt()],
            )
            for t in range(T):
                g = gather_tile(table1, t, "g1")
                pss = [
                    psA.tile([P, P], F32, tag=f"agg{h}", name=f"agg{h}")
                    for h in range(2)
                ]
                for k in range(C):
                    s_t = selector(t, k)
                    for h in range(2):
                        # out[feat_h, dest] += G[:, k, feat_h]^T @ S
                        nc.tensor.matmul(
                            pss[h],
                            lhsT=g[:, k, h * P : (h + 1) * P],
                            rhs=s_t[:],
                            start=(k == 0),
                            stop=(k == C - 1),
                        )
                for h in range(2):
                    # hT[feat_h, t*128:...] = relu(agg + b1[feat_h])
                    nc.scalar.activation(
                        hT[:, h * SHARD + t * P : h * SHARD + (t + 1) * P],
                        pss[h][:],
                        mybir.ActivationFunctionType.Relu,
                        bias=b1[:, h : h + 1],
                    )

            # ---------- layer 2 ----------
            dense_support(hT, w2, cc2_in)
            nc.gpsimd.collective_compute(
                "AllGather",
                mybir.AluOpType.bypass,
                replica_groups=rg,
                ins=[cc2_in.opt()],
                outs=[table2.opt()],
            )
            for t in range(T):
                g = gather_tile(table2, t, "g2")
                ps = psD.tile([P, D], F32, tag="psD", name="agg2")
                for k in range(C):
                    s_t = selector(t, k)
                    # out[dest, :] += S^T @ G[:, k, :]
                    nc.tensor.matmul(
                        ps,
                        lhsT=s_t[:],
                        rhs=g[:, k, :],
                        start=(k == 0),
                        stop=(k == C - 1),
                    )
                nc.vector.tensor_tensor(ps[:], ps[:], b2[:], mybir.AluOpType.add)
                ot = dense.tile([P, D], F32, tag="ot", name="ot")
                nc.scalar.activation(
                    ot[:], ps[:], mybir.ActivationFunctionType.Relu
                )
                nc.sync.dma_start(out_d[ts(t, P), :], ot[:])

    nc.compile()
    _cache[key] = nc
    return nc


def _wrap_idx16(flat: np.ndarray) -> np.ndarray:
    """[L] int -> [128, L/16] int16 SBUF wrap: sb[p, s] = flat[s*16 + p%16]."""
    L = flat.shape[0]
    base = flat.reshape(L // 16, 16).T.astype(np.int16)  # [16, L/16]
    return np.tile(base, (8, 1))


def _preprocess(adj_rows, adj_cols, adj_vals):
    r = np.asarray(adj_rows).astype(np.int64)
    c = np.asarray(adj_cols).astype(np.int64)
    v = np.asarray(adj_vals).astype(np.float32)
    E = r.shape[0]

    core = r // SHARD
    tile_id = (r % SHARD) // P
    dest_local = (r % P).astype(np.float32)
    half = (c >= HALF).astype(np.int64)
    idx_local = c - half * HALF

    key = (core * T + tile_id) * 2 + half
    order = np.lexsort((c, key))
    key_s = key[order]
    counts = np.bincount(key_s, minlength=NC * T * 2)
    grp_start = np.concatenate(([0], np.cumsum(counts)))[:-1]
    j = np.arange(E) - grp_start[key_s]

    cnt = counts.reshape(NC, T, 2)
    C0 = max(1, -(-int(cnt[:, :, 0].max()) // P))
    C1 = -(-int(cnt[:, :, 1].max()) // P)
    C = C0 + C1

    core_s = core[order]
    tile_s = tile_id[order]
    half_s = half[order]
    chunk = j // P + np.where(half_s == 1, C0, 0)
    part = j % P

    idx_pad = np.zeros((NC, T, C, P), np.int16)
    dest_pad = np.zeros((NC, T, C, P), np.float32)
    val_pad = np.zeros((NC, T, C, P), np.float32)
    idx_pad[core_s, tile_s, chunk, part] = idx_local[order].astype(np.int16)
    dest_pad[core_s, tile_s, chunk, part] = dest_local[order]
    val_pad[core_s, tile_s, chunk, part] = v[order]

    # gidx: per (core, tile): C0*8 cols of wrapped group-0 idxs, then C1*8
    gidx = np.zeros((NC, P, T * C * 8), np.int16)
    for cr in range(NC):
        for t in range(T):
            base = t * C * 8
            g0 = idx_pad[cr, t, 0:C0, :].reshape(C0 * P)
            gidx[cr, :, base : base + C0 * 8] = _wrap_idx16(g0)
            if C1 > 0:
                g1 = idx_pad[cr, t, C0:C, :].reshape(C1 * P)
                gidx[cr, :, base + C0 * 8 : base + C * 8] = _wrap_idx16(g1)

    # meta: [P, T*2C] with cols t*2C + 2k = dest_local, +1 = val
    meta = np.empty((NC, P, T, C, 2), np.float32)
    meta[..., 0] = dest_pad.transpose(0, 3, 1, 2)
    meta[..., 1] = val_pad.transpose(0, 3, 1, 2)
    meta = meta.reshape(NC, P, T * C * 2)

    return C0, C1, gidx, meta


def kernel(
    x, adj_rows, adj_cols, adj_vals, pad_n, pos_idx, W1, b1, W2, b2
) -> np.ndarray:
    x = np.asarray(x, np.float32)
    W1 = np.asarray(W1, np.float32)
    b1 = np.asarray(b1, np.float32)
    W2 = np.asarray(W2, np.float32)
    b2 = np.asarray(b2, np.float32)
    pos_idx = np.asarray(pos_idx).astype(np.int64)
    pad_n_i = int(pad_n)
    assert x.shape == (N, D)

    C0, C1, gidx, meta = _preprocess(adj_rows, adj_cols, adj_vals)
    nc = _build(C0, C1)

    xpad = np.zeros((NPAD, D), np.float32)
    xpad[:N] = x
    w1h = W1.astype(np.float16).reshape(2, P, D)
    w2h = W2.astype(np.float16).reshape(2, P, D)
    b1c = np.ascontiguousarray(b1.reshape(2, P).T.astype(np.float32))
    b2b = np.ascontiguousarray(np.broadcast_to(b2, (P, D)).astype(np.float32))
    iota128 = np.ascontiguousarray(
        np.broadcast_to(np.arange(P, dtype=np.float16), (P, P))
    )

    in_maps = []
    for cr in range(NC):
        xT = np.ascontiguousarray(
            xpad[cr * SHARD : (cr + 1) * SHARD].T.astype(np.float16).reshape(2, P, SHARD)
        )
        in_maps.append(
            {
                "xT": xT,
                "W1h": w1h,
                "W2h": w2h,
                "b1c": b1c,
                "b2b": b2b,
                "iota128": iota128,
                "gidx": np.ascontiguousarray(gidx[cr]),
                "meta": np.ascontiguousarray(meta[cr]),
            }
        )

    trace = bool(int(os.environ.get("KERNEL_TRACE", "0")))
    res = None
    for attempt in range(3):
        try:
            res = bass_utils.run_bass_kernel_spmd(
                nc, in_maps, core_ids=list(range(NC)), trace=trace
            )
            break
        except Exception:
            if attempt == 2:
                raise
            import time as _time

            _time.sleep(10.0)
    global last_results
    last_results = res

    h2 = np.concatenate([res.results[cr]["out"] for cr in range(NC)], axis=0)[:N]
    out = np.zeros((pad_n_i, D), np.float32)
    out[pos_idx] = h2
    return out


# revision 11
# speedup vs baseline: 2.0101x; 1.2292x over previous
"""GCN encoder (2-layer graph conv) on 8 Trainium2 NeuronCores.

Strategy (sharding_hint): nodes sharded across the 8 cores by destination row
(contiguous blocks of 6272 padded rows); edges partitioned by destination;
256x256 weights replicated. Per layer:

  1. each core computes its shard of support = x @ W (dense matmul, fp16),
  2. AllGather -> full fp16 "support table" [50176, 256] in every core's HBM,
  3. per 128-row destination tile: dma_gather the source rows for its edges
     (sorted by dest; sub-grouped by source range so int16 indices fit;
     <=1024 indices per gather, round-robined over 4 SWDGE queues),
     and accumulate agg += selector-matmuls into PSUM (segment-sum as matmul,
     contracting over 128-edge chunks). The scaled one-hot selector matrices
     S[e, dest_local] = val_e are precomputed on the host and streamed in by
     plain HWDGE DMA - no on-device selector construction.
  4. epilogue relu(agg + b) on the scalar engine.

Layer 1 runs the selector matmul "transposed" (out = G_half^T @ S -> [feat,
dest]) so the result lands as hT, which feeds layer 2's support matmul as the
stationary operand without any transposes. Layer 2 runs it straight
(out = S^T @ G -> [dest, feat]) so the final output is row-major.

Host-side work is only index/format preprocessing (sort edges, build the
padded chunk metadata and selector tensors) and the final unshard/pad via
pos_idx.
"""

import os
import sys

if "/opt/trn_rl_repo" not in sys.path:
    sys.path.insert(0, "/opt/trn_rl_repo")

import numpy as np

import concourse.mybir as mybir
import concourse.tile as tile
from concourse import bacc, bass_utils
from concourse.bass import ts
from concourse.library_config import mlp

# Problem geometry (nn_GCNEncoder: N=50000, E=1.6M, 256 features, pad to 60000)
N = 50000
D = 256
NC = 8
P = 128
T = 49  # dest-row tiles per core
SHARD = T * P  # 6272 rows per core
NPAD = NC * SHARD  # 50176
HALFA = 32768  # int16 gather indices: sources < 32768 vs [32768, 50176)
HALFB = NPAD - HALFA  # 17408

F16 = mybir.dt.float16
F32 = mybir.dt.float32
I16 = mybir.dt.int16

_cache: dict = {}
last_results = None  # BassKernelResults of the most recent run (for profiling)


def _build(c0t: tuple, c1t: tuple):
    """Build + compile the SPMD program.

    c0t/c1t: per-dest-tile chunk counts (chunks of 128 edges) for the two
    source-index groups ([0, 32768) and [32768, 50176))."""
    key = (c0t, c1t)
    if key in _cache:
        return _cache[key]

    ct = [a + b for a, b in zip(c0t, c1t)]
    off = np.concatenate(([0], np.cumsum(ct))).astype(int)  # chunk offsets
    TOTC = int(off[-1])
    CMAX = max(ct)

    nc = bacc.Bacc(
        "TRN2",
        target_bir_lowering=False,
        debug=False,
        num_devices=NC,
        num_swdge_queues=4,
    )

    xT_d = nc.dram_tensor("xT", [2, P, SHARD], F16, kind="ExternalInput")
    w1_d = nc.dram_tensor("W1h", [2, P, D], F16, kind="ExternalInput")
    w2_d = nc.dram_tensor("W2h", [2, P, D], F16, kind="ExternalInput")
    b1_d = nc.dram_tensor("b1c", [P, 2], F32, kind="ExternalInput")
    b2_d = nc.dram_tensor("b2b", [P, D], F32, kind="ExternalInput")
    gidx_d = nc.dram_tensor("gidx", [P, TOTC * 8], I16, kind="ExternalInput")
    sel_d = nc.dram_tensor("sel", [P, TOTC * P], F16, kind="ExternalInput")
    out_d = nc.dram_tensor("out", [SHARD, D], F32, kind="ExternalOutput")

    nc.gpsimd.load_library(mlp)

    rg = [list(range(NC))]

    with tile.TileContext(nc) as tc:
        with (
            tc.tile_pool(name="const", bufs=1) as const,
            tc.tile_pool(name="gpool", bufs=2) as gpool,
            tc.tile_pool(name="spool", bufs=2) as spool,
            tc.tile_pool(name="dense", bufs=3) as dense,
            tc.tile_pool(name="psD", bufs=2, space="PSUM") as psD,
            tc.tile_pool(name="psA", bufs=2, space="PSUM") as psA,
            tc.tile_pool(name="dram", bufs=1, space="DRAM") as dram,
        ):
            cc1_in = dram.tile([SHARD, D], F16)
            table1 = dram.tile([NPAD, D], F16, addr_space="Shared")
            cc2_in = dram.tile([SHARD, D], F16)
            table2 = dram.tile([NPAD, D], F16, addr_space="Shared")

            # --- persistent SBUF state ---
            gidx = const.tile([P, TOTC * 8], I16)
            nc.sync.dma_start(gidx[:], gidx_d[:])
            b1 = const.tile([P, 2], F32)
            nc.sync.dma_start(b1[:], b1_d[:])
            b2 = const.tile([P, D], F32)
            nc.sync.dma_start(b2[:], b2_d[:])
            w1 = const.tile([P, 2 * D], F16)
            w2 = const.tile([P, 2 * D], F16)
            # xsb and hT share one slot: xsb's last read (layer-1 dense
            # matmuls) precedes hT's first write (layer-1 gather epilogue)
            xsb = const.tile([P, 2 * SHARD], F16, tag="xht", name="xsb")
            hT = const.tile([P, 2 * SHARD], F16, tag="xht", name="hT")
            for h in range(2):
                nc.sync.dma_start(w1[:, h * D : (h + 1) * D], w1_d[h])
                nc.sync.dma_start(w2[:, h * D : (h + 1) * D], w2_d[h])
                nc.sync.dma_start(xsb[:, h * SHARD : (h + 1) * SHARD], xT_d[h])

            def dense_support(src_sb, w_sb, cc_in):
                # support[rows, :] = x[rows, :] @ W  (K=256 split into 2 halves)
                for t in range(T):
                    ps = psD.tile([P, D], F32, tag="psD", name="ps_dense")
                    for h in range(2):
                        nc.tensor.matmul(
                            ps,
                            lhsT=src_sb[:, h * SHARD + t * P : h * SHARD + (t + 1) * P],
                            rhs=w_sb[:, h * D : (h + 1) * D],
                            start=(h == 0),
                            stop=(h == 1),
                        )
                    st = dense.tile([P, D], F16, tag="stage", name="stage")
                    nc.scalar.copy(st[:], ps[:])
                    nc.sync.dma_start(cc_in[ts(t, P), :], st[:])

            gather_counter = [0]

            def gather_tile(table, t, tag):
                # dma_gather hard-crashes above 1024 indices -> <=8 chunks per
                # op; round-robin across the 4 SWDGE queues
                c0, c1, c = c0t[t], c1t[t], ct[t]
                g = gpool.tile([P, CMAX, D], F16, tag=tag, name="gtile")
                base = int(off[t]) * 8
                for lo, hi, toff, tsz in (
                    (0, c0, 0, HALFA),
                    (c0, c, HALFA, HALFB),
                ):
                    k = lo
                    while k < hi:
                        kk = min(8, hi - k)
                        nc.gpsimd.dma_gather(
                            g[:, k : k + kk, :],
                            table[toff : toff + tsz, :],
                            gidx[:, base + k * 8 : base + (k + kk) * 8],
                            num_idxs=kk * P,
                            num_idxs_reg=kk * P,
                            elem_size=D,
                            queue_num=gather_counter[0] % 4,
                        )
                        gather_counter[0] += 1
                        k += kk
                return g

            def sel_tile(t):
                c = ct[t]
                s = spool.tile([P, CMAX * P], F16, tag="sel", name="sel")
                nc.sync.dma_start(
                    s[:, : c * P],
                    sel_d[:, int(off[t]) * P : (int(off[t]) + c) * P],
                )
                return s

            # ---------- layer 1 ----------
            dense_support(xsb, w1, cc1_in)
            nc.gpsimd.collective_compute(
                "AllGather",
                mybir.AluOpType.bypass,
                replica_groups=rg,
                ins=[cc1_in.opt()],
                outs=[table1.opt()],
            )
            for t in range(T):
                c = ct[t]
                g = gather_tile(table1, t, "g1")
                s = sel_tile(t)
                pss = [
                    psA.tile([P, P], F32, tag=f"agg{h}", name=f"agg{h}")
                    for h in range(2)
                ]
                for k in range(c):
                    for h in range(2):
                        # out[feat_h, dest] += G[:, k, feat_h]^T @ S_k
                        nc.tensor.matmul(
                            pss[h],
                            lhsT=g[:, k, h * P : (h + 1) * P],
                            rhs=s[:, k * P : (k + 1) * P],
                            start=(k == 0),
                            stop=(k == c - 1),
                        )
                for h in range(2):
                    # hT[feat_h, t*128:...] = relu(agg + b1[feat_h])
                    nc.scalar.activation(
                        hT[:, h * SHARD + t * P : h * SHARD + (t + 1) * P],
                        pss[h][:],
                        mybir.ActivationFunctionType.Relu,
                        bias=b1[:, h : h + 1],
                    )

            # ---------- layer 2 ----------
            dense_support(hT, w2, cc2_in)
            nc.gpsimd.collective_compute(
                "AllGather",
                mybir.AluOpType.bypass,
                replica_groups=rg,
                ins=[cc2_in.opt()],
                outs=[table2.opt()],
            )
            for t in range(T):
                c = ct[t]
                g = gather_tile(table2, t, "g2")
                s = sel_tile(t)
                ps = psD.tile([P, D], F32, tag="psD", name="agg2")
                for k in range(c):
                    # out[dest, :] += S_k^T @ G[:, k, :]
                    nc.tensor.matmul(
                        ps,
                        lhsT=s[:, k * P : (k + 1) * P],
                        rhs=g[:, k, :],
                        start=(k == 0),
                        stop=(k == c - 1),
                    )
                nc.vector.tensor_tensor(ps[:], ps[:], b2[:], mybir.AluOpType.add)
                ot = dense.tile([P, D], F32, tag="ot", name="ot")
                nc.scalar.activation(ot[:], ps[:], mybir.ActivationFunctionType.Relu)
                nc.sync.dma_start(out_d[ts(t, P), :], ot[:])

    nc.compile()
    _cache[key] = nc
    return nc


def _wrap_idx16(flat: np.ndarray) -> np.ndarray:
    """[L] int -> [128, L/16] int16 SBUF wrap: sb[p, s] = flat[s*16 + p%16]."""
    L = flat.shape[0]
    base = flat.reshape(L // 16, 16).T.astype(np.int16)  # [16, L/16]
    return np.tile(base, (8, 1))


def _preprocess(adj_rows, adj_cols, adj_vals):
    r = np.asarray(adj_rows).astype(np.int64)
    c = np.asarray(adj_cols).astype(np.int64)
    v = np.asarray(adj_vals).astype(np.float32)
    E = r.shape[0]

    core = r // SHARD
    tile_id = (r % SHARD) // P
    dest_local = r % P
    grp = (c >= HALFA).astype(np.int64)
    idx_local = c - grp * HALFA

    key = (core * T + tile_id) * 2 + grp
    order = np.lexsort((c, key))
    key_s = key[order]
    counts = np.bincount(key_s, minlength=NC * T * 2)
    grp_start = np.concatenate(([0], np.cumsum(counts)))[:-1]
    j = np.arange(E) - grp_start[key_s]

    cnt = counts.reshape(NC, T, 2)  # [core, tile, grp]
    c0t = tuple(
        max(1, -(-int(cnt[:, t, 0].max()) // P)) for t in range(T)
    )
    c1t = tuple(-(-int(cnt[:, t, 1].max()) // P) for t in range(T))
    ct = [a + b for a, b in zip(c0t, c1t)]
    off = np.concatenate(([0], np.cumsum(ct))).astype(int)
    TOTC = int(off[-1])

    core_s = core[order]
    tile_s = tile_id[order]
    grp_s = grp[order]
    c0_arr = np.asarray(c0t)
    chunk_s = j // P + np.where(grp_s == 1, c0_arr[tile_s], 0)
    part_s = j % P
    gchunk_s = off[tile_s] + chunk_s  # global chunk id 0..TOTC-1

    # selector tensors: sel[core][p, gchunk*128 + dest] = val
    sel = np.zeros((NC, P, TOTC * P), np.float16)
    sel[core_s, part_s, gchunk_s * P + dest_local[order]] = v[order]

    # gather indices, wrapped in 16 partitions; idx 0 padding
    idx_pad = np.zeros((NC, TOTC, P), np.int16)
    idx_pad[core_s, gchunk_s, part_s] = idx_local[order].astype(np.int16)
    gidx = np.zeros((NC, P, TOTC * 8), np.int16)
    for cr in range(NC):
        for t in range(T):
            o = int(off[t])
            flat = idx_pad[cr, o : o + ct[t], :].reshape(ct[t] * P)
            gidx[cr, :, o * 8 : (o + ct[t]) * 8] = _wrap_idx16(flat)

    return c0t, c1t, gidx, sel


def kernel(
    x, adj_rows, adj_cols, adj_vals, pad_n, pos_idx, W1, b1, W2, b2
) -> np.ndarray:
    x = np.asarray(x, np.float32)
    W1 = np.asarray(W1, np.float32)
    b1 = np.asarray(b1, np.float32)
    W2 = np.asarray(W2, np.float32)
    b2 = np.asarray(b2, np.float32)
    pos_idx = np.asarray(pos_idx).astype(np.int64)
    pad_n_i = int(pad_n)
    assert x.shape == (N, D)

    c0t, c1t, gidx, sel = _preprocess(adj_rows, adj_cols, adj_vals)
    nc = _build(c0t, c1t)

    xpad = np.zeros((NPAD, D), np.float32)
    xpad[:N] = x
    w1h = W1.astype(np.float16).reshape(2, P, D)
    w2h = W2.astype(np.float16).reshape(2, P, D)
    b1c = np.ascontiguousarray(b1.reshape(2, P).T.astype(np.float32))
    b2b = np.ascontiguousarray(np.broadcast_to(b2, (P, D)).astype(np.float32))

    in_maps = []
    for cr in range(NC):
        xT = np.ascontiguousarray(
            xpad[cr * SHARD : (cr + 1) * SHARD].T.astype(np.float16).reshape(2, P, SHARD)
        )
        in_maps.append(
            {
                "xT": xT,
                "W1h": w1h,
                "W2h": w2h,
                "b1c": b1c,
                "b2b": b2b,
                "gidx": np.ascontiguousarray(gidx[cr]),
                "sel": np.ascontiguousarray(sel[cr]),
            }
        )

    trace = bool(int(os.environ.get("KERNEL_TRACE", "0")))
    res = None
    for attempt in range(3):
        try:
            res = bass_utils.run_bass_kernel_spmd(
                nc, in_maps, core_ids=list(range(NC)), trace=trace
            )
            break
        except Exception:
            if attempt == 2:
                raise
            import time as _time

            _time.sleep(10.0)
    global last_results
    last_results = res

    h2 = np.concatenate([res.results[cr]["out"] for cr in range(NC)], axis=0)[:N]
    out = np.zeros((pad_n_i, D), np.float32)
    out[pos_idx] = h2
    return out


# revision 12
# speedup vs baseline: 2.3594x; 1.1738x over previous
"""GCN encoder (2-layer graph conv) on 8 Trainium2 NeuronCores.

Strategy (sharding_hint): nodes sharded across the 8 cores by destination row
(contiguous blocks of 6272 padded rows); edges partitioned by destination;
256x256 weights replicated. Per layer:

  1. each core computes its shard of support = x @ W (dense matmul, fp16),
  2. two AllGathers (low/high half of each shard) -> full fp16 "support
     tables" in every core's HBM; the split lets each collective start as
     soon as half the dense tiles are staged and lets gathers start after
     only the matching collective finishes,
  3. per 128-row destination tile: dma_gather the source rows for its edges
     (sorted by dest; grouped by source half so int16 indices fit;
     <=1024 indices per gather, round-robined over 4 SWDGE queues),
     and accumulate agg += selector-matmuls into PSUM (segment-sum as
     matmul, contracting over 128-edge chunks). The scaled one-hot selector
     matrices S[e, dest_local] = val_e are precomputed on the host and
     streamed in by plain HWDGE DMA - no on-device selector construction.
  4. epilogue relu(agg + b) on the scalar engine.

Layer 1 runs the selector matmul "transposed" (out = G_half^T @ S -> [feat,
dest]) so the result lands as hT, which feeds layer 2's support matmul as the
stationary operand without any transposes. Layer 2 runs it straight
(out = S^T @ G -> [dest, feat]) so the final output is row-major.

Host-side work is only index/format preprocessing (sort edges, build the
padded chunk metadata and selector tensors) and the final unshard/pad via
pos_idx.
"""

import os
import sys

if "/opt/trn_rl_repo" not in sys.path:
    sys.path.insert(0, "/opt/trn_rl_repo")

import numpy as np

import concourse.mybir as mybir
import concourse.tile as tile
from concourse import bacc, bass_utils
from concourse.bass import ts
from concourse.library_config import mlp

# Problem geometry (nn_GCNEncoder: N=50000, E=1.6M, 256 features, pad to 60000)
N = 50000
D = 256
NC = 8
P = 128
T = 49  # dest-row tiles per core
SHARD = T * P  # 6272 rows per core
NPAD = NC * SHARD  # 50176
LO_T = 25  # dest tiles in the "lo" half of each shard
LO_R = LO_T * P  # 3200
HI_T = T - LO_T  # 24
HI_R = HI_T * P  # 3072
LO_ROWS = NC * LO_R  # 25600 rows in the lo table  (int16-safe)
HI_ROWS = NC * HI_R  # 24576 rows in the hi table

F16 = mybir.dt.float16
F32 = mybir.dt.float32
I16 = mybir.dt.int16

_cache: dict = {}
last_results = None  # BassKernelResults of the most recent run (for profiling)


def _build(c0t: tuple, c1t: tuple):
    """Build + compile the SPMD program.

    c0t/c1t: per-dest-tile chunk counts (chunks of 128 edges) for the two
    source groups (lo-table sources vs hi-table sources)."""
    key = (c0t, c1t)
    if key in _cache:
        return _cache[key]

    ct = [a + b for a, b in zip(c0t, c1t)]
    off = np.concatenate(([0], np.cumsum(ct))).astype(int)  # chunk offsets
    TOTC = int(off[-1])
    CMAX = max(ct)

    nc = bacc.Bacc(
        "TRN2",
        target_bir_lowering=False,
        debug=False,
        num_devices=NC,
        num_swdge_queues=4,
    )

    xT_d = nc.dram_tensor("xT", [2, P, SHARD], F16, kind="ExternalInput")
    w1_d = nc.dram_tensor("W1h", [2, P, D], F16, kind="ExternalInput")
    w2_d = nc.dram_tensor("W2h", [2, P, D], F16, kind="ExternalInput")
    b1_d = nc.dram_tensor("b1c", [P, 2], F32, kind="ExternalInput")
    b2_d = nc.dram_tensor("b2b", [P, D], F32, kind="ExternalInput")
    gidx_d = nc.dram_tensor("gidx", [P, TOTC * 8], I16, kind="ExternalInput")
    sel_d = nc.dram_tensor("sel", [P, TOTC * P], F16, kind="ExternalInput")
    out_d = nc.dram_tensor("out", [SHARD, D], F32, kind="ExternalOutput")

    nc.gpsimd.load_library(mlp)

    rg = [list(range(NC))]

    with tile.TileContext(nc) as tc:
        with (
            tc.tile_pool(name="const", bufs=1) as const,
            tc.tile_pool(name="gpool", bufs=2) as gpool,
            tc.tile_pool(name="spool", bufs=2) as spool,
            tc.tile_pool(name="dense", bufs=3) as dense,
            tc.tile_pool(name="psD", bufs=2, space="PSUM") as psD,
            tc.tile_pool(name="psA", bufs=2, space="PSUM") as psA,
            tc.tile_pool(name="dram", bufs=1, space="DRAM") as dram,
        ):
            cc1_lo = dram.tile([LO_R, D], F16)
            cc1_hi = dram.tile([HI_R, D], F16)
            t1_lo = dram.tile([LO_ROWS, D], F16, addr_space="Shared")
            t1_hi = dram.tile([HI_ROWS, D], F16, addr_space="Shared")
            cc2_lo = dram.tile([LO_R, D], F16)
            cc2_hi = dram.tile([HI_R, D], F16)
            t2_lo = dram.tile([LO_ROWS, D], F16, addr_space="Shared")
            t2_hi = dram.tile([HI_ROWS, D], F16, addr_space="Shared")

            # --- persistent SBUF state ---
            gidx = const.tile([P, TOTC * 8], I16)
            nc.sync.dma_start(gidx[:], gidx_d[:])
            b1 = const.tile([P, 2], F32)
            nc.sync.dma_start(b1[:], b1_d[:])
            b2 = const.tile([P, D], F32)
            nc.sync.dma_start(b2[:], b2_d[:])
            w1 = const.tile([P, 2 * D], F16)
            w2 = const.tile([P, 2 * D], F16)
            # xsb and hT share one slot: xsb's last read (layer-1 dense
            # matmuls) precedes hT's first write (layer-1 gather epilogue)
            xsb = const.tile([P, 2 * SHARD], F16, tag="xht", name="xsb")
            hT = const.tile([P, 2 * SHARD], F16, tag="xht", name="hT")
            for h in range(2):
                nc.sync.dma_start(w1[:, h * D : (h + 1) * D], w1_d[h])
                nc.sync.dma_start(w2[:, h * D : (h + 1) * D], w2_d[h])
                nc.sync.dma_start(xsb[:, h * SHARD : (h + 1) * SHARD], xT_d[h])

            def dense_support(src_sb, w_sb, cc_lo, cc_hi):
                # support[rows, :] = x[rows, :] @ W  (K=256 split into 2 halves)
                for t in range(T):
                    ps = psD.tile([P, D], F32, tag="psD", name="ps_dense")
                    for h in range(2):
                        nc.tensor.matmul(
                            ps,
                            lhsT=src_sb[:, h * SHARD + t * P : h * SHARD + (t + 1) * P],
                            rhs=w_sb[:, h * D : (h + 1) * D],
                            start=(h == 0),
                            stop=(h == 1),
                        )
                    st = dense.tile([P, D], F16, tag="stage", name="stage")
                    nc.scalar.copy(st[:], ps[:])
                    if t < LO_T:
                        nc.sync.dma_start(cc_lo[ts(t, P), :], st[:])
                    else:
                        nc.sync.dma_start(cc_hi[ts(t - LO_T, P), :], st[:])

            def allgathers(cc_lo, cc_hi, t_lo, t_hi):
                nc.gpsimd.collective_compute(
                    "AllGather",
                    mybir.AluOpType.bypass,
                    replica_groups=rg,
                    ins=[cc_lo.opt()],
                    outs=[t_lo.opt()],
                )
                nc.gpsimd.collective_compute(
                    "AllGather",
                    mybir.AluOpType.bypass,
                    replica_groups=rg,
                    ins=[cc_hi.opt()],
                    outs=[t_hi.opt()],
                )

            gather_counter = [0]

            def gather_tile(t_lo, t_hi, t, tag):
                # dma_gather hard-crashes above 1024 indices -> <=8 chunks per
                # op; round-robin across the 4 SWDGE queues
                c0, c1, c = c0t[t], c1t[t], ct[t]
                g = gpool.tile([P, CMAX, D], F16, tag=tag, name="gtile")
                base = int(off[t]) * 8
                for lo, hi, table in ((0, c0, t_lo), (c0, c, t_hi)):
                    k = lo
                    while k < hi:
                        kk = min(8, hi - k)
                        nc.gpsimd.dma_gather(
                            g[:, k : k + kk, :],
                            table[:],
                            gidx[:, base + k * 8 : base + (k + kk) * 8],
                            num_idxs=kk * P,
                            num_idxs_reg=kk * P,
                            elem_size=D,
                            queue_num=gather_counter[0] % 4,
                        )
                        gather_counter[0] += 1
                        k += kk
                return g

            def sel_tile(t):
                c = ct[t]
                s = spool.tile([P, CMAX * P], F16, tag="sel", name="sel")
                nc.sync.dma_start(
                    s[:, : c * P],
                    sel_d[:, int(off[t]) * P : (int(off[t]) + c) * P],
                )
                return s

            # ---------- layer 1 ----------
            dense_support(xsb, w1, cc1_lo, cc1_hi)
            allgathers(cc1_lo, cc1_hi, t1_lo, t1_hi)
            for t in range(T):
                c = ct[t]
                g = gather_tile(t1_lo, t1_hi, t, "g1")
                s = sel_tile(t)
                pss = [
                    psA.tile([P, P], F32, tag=f"agg{h}", name=f"agg{h}")
                    for h in range(2)
                ]
                for k in range(c):
                    for h in range(2):
                        # out[feat_h, dest] += G[:, k, feat_h]^T @ S_k
                        nc.tensor.matmul(
                            pss[h],
                            lhsT=g[:, k, h * P : (h + 1) * P],
                            rhs=s[:, k * P : (k + 1) * P],
                            start=(k == 0),
                            stop=(k == c - 1),
                        )
                for h in range(2):
                    # hT[feat_h, t*128:...] = relu(agg + b1[feat_h])
                    nc.scalar.activation(
                        hT[:, h * SHARD + t * P : h * SHARD + (t + 1) * P],
                        pss[h][:],
                        mybir.ActivationFunctionType.Relu,
                        bias=b1[:, h : h + 1],
                    )

            # ---------- layer 2 ----------
            dense_support(hT, w2, cc2_lo, cc2_hi)
            allgathers(cc2_lo, cc2_hi, t2_lo, t2_hi)
            for t in range(T):
                c = ct[t]
                g = gather_tile(t2_lo, t2_hi, t, "g2")
                s = sel_tile(t)
                ps = psD.tile([P, D], F32, tag="psD", name="agg2")
                for k in range(c):
                    # out[dest, :] += S_k^T @ G[:, k, :]
                    nc.tensor.matmul(
                        ps,
                        lhsT=s[:, k * P : (k + 1) * P],
                        rhs=g[:, k, :],
                        start=(k == 0),
                        stop=(k == c - 1),
                    )
                nc.vector.tensor_tensor(ps[:], ps[:], b2[:], mybir.AluOpType.add)
                ot = dense.tile([P, D], F32, tag="ot", name="ot")
                nc.scalar.activation(ot[:], ps[:], mybir.ActivationFunctionType.Relu)
                nc.sync.dma_start(out_d[ts(t, P), :], ot[:])

    nc.compile()
    _cache[key] = nc
    return nc


def _wrap_idx16(flat: np.ndarray) -> np.ndarray:
    """[L] int -> [128, L/16] int16 SBUF wrap: sb[p, s] = flat[s*16 + p%16]."""
    L = flat.shape[0]
    base = flat.reshape(L // 16, 16).T.astype(np.int16)  # [16, L/16]
    return np.tile(base, (8, 1))


def _preprocess(adj_rows, adj_cols, adj_vals):
    r = np.asarray(adj_rows).astype(np.int64)
    c = np.asarray(adj_cols).astype(np.int64)
    v = np.asarray(adj_vals).astype(np.float32)
    E = r.shape[0]

    core = r // SHARD
    tile_id = (r % SHARD) // P
    dest_local = r % P
    # source -> (lo/hi table, local index)
    s_core = c // SHARD
    s_loc = c % SHARD
    grp = (s_loc >= LO_R).astype(np.int64)
    idx_local = np.where(grp == 0, s_core * LO_R + s_loc, s_core * HI_R + s_loc - LO_R)

    key = (core * T + tile_id) * 2 + grp
    order = np.lexsort((c, key))
    key_s = key[order]
    counts = np.bincount(key_s, minlength=NC * T * 2)
    grp_start = np.concatenate(([0], np.cumsum(counts)))[:-1]
    j = np.arange(E) - grp_start[key_s]

    cnt = counts.reshape(NC, T, 2)  # [core, tile, grp]
    c0t = tuple(max(1, -(-int(cnt[:, t, 0].max()) // P)) for t in range(T))
    c1t = tuple(max(1, -(-int(cnt[:, t, 1].max()) // P)) for t in range(T))
    ct = [a + b for a, b in zip(c0t, c1t)]
    off = np.concatenate(([0], np.cumsum(ct))).astype(int)
    TOTC = int(off[-1])

    core_s = core[order]
    tile_s = tile_id[order]
    grp_s = grp[order]
    c0_arr = np.asarray(c0t)
    chunk_s = j // P + np.where(grp_s == 1, c0_arr[tile_s], 0)
    part_s = j % P
    gchunk_s = off[tile_s] + chunk_s  # global chunk id 0..TOTC-1

    # selector tensors: sel[core][p, gchunk*128 + dest] = val
    sel = np.zeros((NC, P, TOTC * P), np.float16)
    sel[core_s, part_s, gchunk_s * P + dest_local[order]] = v[order]

    # gather indices, wrapped in 16 partitions; idx 0 padding
    idx_pad = np.zeros((NC, TOTC, P), np.int16)
    idx_pad[core_s, gchunk_s, part_s] = idx_local[order].astype(np.int16)
    gidx = np.zeros((NC, P, TOTC * 8), np.int16)
    for cr in range(NC):
        for t in range(T):
            o = int(off[t])
            flat = idx_pad[cr, o : o + ct[t], :].reshape(ct[t] * P)
            gidx[cr, :, o * 8 : (o + ct[t]) * 8] = _wrap_idx16(flat)

    return c0t, c1t, gidx, sel


def kernel(
    x, adj_rows, adj_cols, adj_vals, pad_n, pos_idx, W1, b1, W2, b2
) -> np.ndarray:
    x = np.asarray(x, np.float32)
    W1 = np.asarray(W1, np.float32)
    b1 = np.asarray(b1, np.float32)
    W2 = np.asarray(W2, np.float32)
    b2 = np.asarray(b2, np.float32)
    pos_idx = np.asarray(pos_idx).astype(np.int64)
    pad_n_i = int(pad_n)
    assert x.shape == (N, D)

    c0t, c1t, gidx, sel = _preprocess(adj_rows, adj_cols, adj_vals)
    nc = _build(c0t, c1t)

    xpad = np.zeros((NPAD, D), np.float32)
    xpad[:N] = x
    w1h = W1.astype(np.float16).reshape(2, P, D)
    w2h = W2.astype(np.float16).reshape(2, P, D)
    b1c = np.ascontiguousarray(b1.reshape(2, P).T.astype(np.float32))
    b2b = np.ascontiguousarray(np.broadcast_to(b2, (P, D)).astype(np.float32))

    in_maps = []
    for cr in range(NC):
        xT = np.ascontiguousarray(
            xpad[cr * SHARD : (cr + 1) * SHARD].T.astype(np.float16).reshape(2, P, SHARD)
        )
        in_maps.append(
            {
                "xT": xT,
                "W1h": w1h,
                "W2h": w2h,
                "b1c": b1c,
                "b2b": b2b,
                "gidx": np.ascontiguousarray(gidx[cr]),
                "sel": np.ascontiguousarray(sel[cr]),
            }
        )

    trace = bool(int(os.environ.get("KERNEL_TRACE", "0")))
    res = None
    for attempt in range(3):
        try:
            res = bass_utils.run_bass_kernel_spmd(
                nc, in_maps, core_ids=list(range(NC)), trace=trace
            )
            break
        except Exception:
            if attempt == 2:
                raise
            import time as _time

            _time.sleep(10.0)
    global last_results
    last_results = res

    h2 = np.concatenate([res.results[cr]["out"] for cr in range(NC)], axis=0)[:N]
    out = np.zeros((pad_n_i, D), np.float32)
    out[pos_idx] = h2
    return out
